# revision 17
# baseline (speedup 1.0000x reference)
"""CrossSharedUnit Trainium2 kernel — 8-core data-parallel over batch.

Reference computation (per batch b, S=128 tokens, H=512 hidden, K=8):
  proj[b,s,k,g] = sum_h left[b,s,h] * G[h,k,g]
  raw[b,s,t,k]  = tanh(sum_g proj[b,s,k,g] * right[b,t,g])
  score[b,s,t]  = sum_k raw[b,s,t,k] * v[k]
  attn          = softmax(score, axis=t)
  out           = self + attn @ other_hidden
for two branches (aspect: left=aspect, right=polarity; polarity: left=aspect,
right=aspect — faithful to the source which uses aspect on BOTH sides).

Sharding: batch B=32 split 4-per-core across 8 cores; G tensors replicated.
No collectives.

Schedule: the PE (tensor engine) is the bottleneck (~78us of fp32r matmul),
so the program is one continuous PE stream:
  warmup | br0-s1 (+ br0-s2-ck0 spliced at k=3) | br0-s2-ck1
         | br1-s1 (+ br1-s2-ck0 splice + br0 z/out mms interleaved)
         | br1-s2-ck1 | br1 z/out
with softmax chains on vector/gpsimd/scalar underneath the next phase's
matmuls. All input DMA issues live on the sync queue in exact consumption
order (plus two tiny v-loads on gpsimd), so a blocked G prefetch (bufs=1
buffer recycling between branches) can never head-of-line-block a PSUM
evacuation; output stores ride the sync queue after the loads are done.

All matmuls are float32r (TF32-like, 1 row/cycle at free>=256). Softmax
needs no max-subtraction: |score| <= sum|v_k| so exp() cannot overflow in
fp32. The softmax division is deferred through the attention matmul:
out = self + (E @ other) / Z with Z from a ones-matmul.
"""

import os
import sys

sys.path.insert(0, "/opt/trn_rl_repo")

import numpy as np

from concourse import bacc, mybir, tile
from concourse.bass_utils import run_bass_kernel_spmd

B, S, H, K = 32, 128, 512, 8
NCORES = 8
BL = B // NCORES          # batches per core
BS = BL * S               # rows per core (512)
P = 128                   # partitions
HT = H // P               # h partition-tiles (4)
KG = K * H                # flattened (k,g) axis (4096)
KC = K // 2               # k's per stage-2 chunk (4)
F32 = mybir.dt.float32
F32R = mybir.dt.float32r

_cache = {}


def _build():
    """Build + compile the per-core Bass program (same program on all cores)."""
    nc = bacc.Bacc("TRN2", target_bir_lowering=False, debug=False,
                   num_devices=NCORES)

    xa_t_d = nc.dram_tensor("xa_t", [P, HT * BS], F32R, kind="ExternalInput")
    xp_t_d = nc.dram_tensor("xp_t", [P, HT * BS], F32R, kind="ExternalInput")
    xa_nat_d = nc.dram_tensor("xa_nat", [P, BL * H], F32R, kind="ExternalInput")
    xp_nat_d = nc.dram_tensor("xp_nat", [P, BL * H], F32R, kind="ExternalInput")
    # G: one pre-shuffled tensor per branch; pieces are column slices in
    # consumption order: 4x k0 per-h, 2x k1 halves, 6x per-k
    g_ap_d = nc.dram_tensor("g_ap", [P, HT * KG], F32R, kind="ExternalInput")
    g_pa_d = nc.dram_tensor("g_pa", [P, HT * KG], F32R, kind="ExternalInput")
    v_ap_d = nc.dram_tensor("v_ap", [K, 1], F32, kind="ExternalInput")
    v_pa_d = nc.dram_tensor("v_pa", [K, 1], F32, kind="ExternalInput")
    out_a_d = nc.dram_tensor("out_a", [BS, H], F32, kind="ExternalOutput")
    out_p_d = nc.dram_tensor("out_p", [BS, H], F32, kind="ExternalOutput")

    Tanh = mybir.ActivationFunctionType.Tanh
    Exp = mybir.ActivationFunctionType.Exp
    MULT = mybir.AluOpType.mult
    ADD = mybir.AluOpType.add

    with tile.TileContext(nc) as tc:
        with (
            tc.tile_pool(name="const", bufs=1) as cpool,
            tc.tile_pool(name="g", bufs=1) as gpool,
            tc.tile_pool(name="proj", bufs=1) as projpool,
            tc.tile_pool(name="work", bufs=2) as work,
            tc.tile_pool(name="ps_acc", bufs=4, space="PSUM") as ps_acc,
            tc.tile_pool(name="ps_o", bufs=2, space="PSUM") as ps_o,
            tc.tile_pool(name="ps_z", bufs=2, space="PSUM") as ps_z,
        ):
            # ---- constants + warmup weights (vector queue) --------------
            wm = cpool.tile([P, BS], F32R, tag="wm")
            nc.vector.memset(wm[:].bitcast(F32), 0.0)
            ones_t = cpool.tile([P, 2], F32R, tag="ones_t")
            nc.vector.memset(ones_t[:].bitcast(F32), 1.0)

            # ---- persistent activations --------------------------------
            xa_t0 = cpool.tile([P, BS], F32R, tag="xa_t0")
            xa_tb = cpool.tile([P, (HT - 1) * BS], F32R, tag="xa_tb")
            xp_t = cpool.tile([P, HT * BS], F32R, tag="xp_t")
            xa_nat = cpool.tile([P, BL * H], F32R, tag="xa_nat")
            xp_nat = cpool.tile([P, BL * H], F32R, tag="xp_nat")

            vrow_a = cpool.tile([1, K], F32, tag="vrow_a")
            vrow_p = cpool.tile([1, K], F32, tag="vrow_p")
            nc.gpsimd.dma_start(out=vrow_a[:], in_=v_ap_d.ap().rearrange("k o -> o k"))
            nc.gpsimd.dma_start(out=vrow_p[:], in_=v_pa_d.ap().rearrange("k o -> o k"))
            vbc_a = cpool.tile([P, K], F32, tag="vbc_a")
            vbc_p = cpool.tile([P, K], F32, tag="vbc_p")
            nc.gpsimd.partition_broadcast(vbc_a[:], vrow_a[:])
            nc.gpsimd.partition_broadcast(vbc_p[:], vrow_p[:])

            # ---- G piece tiles (shared bufs=1 between branches) ---------
            g_k0 = [gpool.tile([P, H], F32R, tag=f"g_k0_{h}",
                                name=f"g_k0_{h}") for h in range(HT)]
            g_k1a = gpool.tile([P, 2 * H], F32R, tag="g_k1a")
            g_k1b = gpool.tile([P, 2 * H], F32R, tag="g_k1b")
            g_kk = [gpool.tile([P, HT * H], F32R, tag=f"g_kk{i}",
                               name=f"g_kk{i}")
                     for i in range(6)]

            # ---- the loader. Host arrays are pre-shuffled partition-major
            # so every DMA is 128 descriptors of one big contiguous chunk.
            # The startup-critical pieces (k0 G + aspect) are 256KB each and
            # spread over three queues for DMA-engine concurrency.
            def load_g_head(g_d):
                for h in range(HT):
                    nc.sync.dma_start(out=g_k0[h][:],
                                      in_=g_d.ap()[:, h * H:(h + 1) * H])
                nc.sync.dma_start(out=g_k1a[:],
                                  in_=g_d.ap()[:, HT * H:HT * H + 2 * H])
                nc.sync.dma_start(out=g_k1b[:],
                                  in_=g_d.ap()[:, HT * H + 2 * H:2 * HT * H])

            def load_g_k(g_d, i):
                o = (2 + i) * HT * H
                nc.sync.dma_start(out=g_kk[i][:], in_=g_d.ap()[:, o:o + HT * H])

            # interleave xa-h / G-k0-h so the PE can start after 512KB and
            # gets one h-step per ~0.8us thereafter
            nc.sync.dma_start(out=xa_t0[:], in_=xa_t_d.ap()[:, 0:BS])
            nc.sync.dma_start(out=g_k0[0][:], in_=g_ap_d.ap()[:, 0:H])
            for h in range(1, HT):
                nc.sync.dma_start(out=xa_tb[:, (h - 1) * BS:h * BS],
                                  in_=xa_t_d.ap()[:, h * BS:(h + 1) * BS])
                nc.sync.dma_start(out=g_k0[h][:],
                                  in_=g_ap_d.ap()[:, h * H:(h + 1) * H])
            nc.sync.dma_start(out=g_k1a[:],
                              in_=g_ap_d.ap()[:, HT * H:HT * H + 2 * H])
            nc.sync.dma_start(out=g_k1b[:],
                              in_=g_ap_d.ap()[:, HT * H + 2 * H:2 * HT * H])
            for i in range(4):
                load_g_k(g_ap_d, i)
            nc.sync.dma_start(out=xp_t[:], in_=xp_t_d.ap()[:])
            load_g_k(g_ap_d, 4)
            load_g_k(g_ap_d, 5)
            nc.sync.dma_start(out=xp_nat[:], in_=xp_nat_d.ap()[:])
            # (g_pa loads are emitted after br0-s1 so the WAR deps pick up
            #  br0's reads; xa_nat after those.)

            def g_lhsT(k, h, gt):
                if k == 0:
                    return g_k0[h][:, gt * P:(gt + 1) * P]
                if k == 1:
                    piece = g_k1a if h < 2 else g_k1b
                    o = (h % 2) * H + gt * P
                    return piece[:, o:o + P]
                piece = g_kk[k - 2]
                o = h * H + gt * P
                return piece[:, o:o + P]

            def xa_rhs(h):
                if h == 0:
                    return xa_t0[:]
                return xa_tb[:, (h - 1) * BS:h * BS]

            def xa_lhsT(gi, b):
                if gi == 0:
                    return xa_t0[:, b * S:(b + 1) * S]
                o = (gi - 1) * BS + b * S
                return xa_tb[:, o:o + S]

            def xp_lhsT(gi, b):
                o = gi * BS + b * S
                return xp_t[:, o:o + S]

            # projT2[gt][g_part, k, b, s] — stage-1 output, stage-2 rhs.
            projT2 = [projpool.tile([P, K, BL, S], F32R, tag=f"projT2_{gt}",
                                    name=f"projT2_{gt}")
                      for gt in range(HT)]

            evac_state = [0]

            def evac(dst, src):
                # ping-pong PSUM evacuations between vector and scalar
                if evac_state[0] % 2 == 0:
                    nc.vector.tensor_copy(dst, src)
                else:
                    nc.scalar.copy(dst, src)
                evac_state[0] += 1

            # ---- PE warmup: get the p-state ramp going during DMA lead-in
            for w in range(5):
                acc = ps_acc.tile([P, BL, S], F32, tag="acc", name=f"warm{w}")
                nc.tensor.matmul(acc[:], wm[:, 0:P], wm[:],
                                 start=True, stop=True)

            def stage1(br):
                # k0 h-outer with 4 open accumulators: first matmuls need
                # only g_k0h0 + xa_t0 (512KB total).
                accs = [ps_acc.tile([P, BL, S], F32, tag="acc",
                                    name=f"s1a{br}k0g{gt}")
                        for gt in range(HT)]
                for h in range(HT):
                    for gt in range(HT):
                        nc.tensor.matmul(
                            accs[gt][:], g_lhsT(0, h, gt), xa_rhs(h),
                            start=(h == 0), stop=(h == HT - 1),
                            skip_group_check=True)
                for gt in range(HT):
                    evac(projT2[gt][:, 0, :, :], accs[gt][:])
                for k in range(1, K):
                    for gt in range(HT):
                        acc = ps_acc.tile([P, BL, S], F32, tag="acc",
                                          name=f"s1a{br}k{k}g{gt}")
                        for h in range(HT):
                            nc.tensor.matmul(
                                acc[:], g_lhsT(k, h, gt), xa_rhs(h),
                                start=(h == 0), stop=(h == HT - 1))
                        evac(projT2[gt][:, k, :, :], acc[:])
                    yield k

            # th_all[t_part, k, b, s]: tanh(stage-2) output, both branches
            # (WAR-recycled). Score ops slice [:, j, :, :] batched over b.
            th_all = work.tile([P, K, BL, S], F32, tag="th", bufs=1)

            def stage2_ck(br, lhsT_of, ck):
                # raw[t, k, s] = tanh(sum_g right[t,g] proj[g,k,s]) per batch
                for b in range(BL):
                    acc2 = ps_acc.tile([P, KC, S], F32, tag="acc",
                                       name=f"s2a{br}b{b}c{ck}")
                    for gi in range(HT):
                        nc.tensor.matmul(
                            acc2[:],
                            lhsT_of(gi, b),
                            projT2[gi][:, ck * KC:(ck + 1) * KC, b, :],
                            start=(gi == 0), stop=(gi == HT - 1))
                    nc.scalar.activation(
                        th_all[:, ck * KC:(ck + 1) * KC, b, :], acc2[:], Tanh)

            def sca_all(vbc):
                # first-half score partial, batched over all 4 batches
                sca = work.tile([P, BL, S], F32, tag="sca")
                nc.vector.tensor_scalar_mul(sca[:], th_all[:, 0, :, :],
                                            vbc[:, 0:1])
                for j in range(1, KC):
                    nc.vector.scalar_tensor_tensor(
                        sca[:], th_all[:, j, :, :], vbc[:, j:j + 1], sca[:],
                        MULT, ADD)
                return sca

            def zout(br, b, e_t, nat_other, nat_self, out_d):
                # out = self + (E_T.T @ other) / Z, Z via ones-matmul.
                zp = ps_z.tile([P, 2], F32, tag="z", name=f"z{br}b{b}")
                nc.tensor.matmul(zp[:], e_t[:, b, :], ones_t[:],
                                 start=True, stop=True)
                rz = work.tile([P, 1], F32, tag="rz", bufs=4)
                nc.vector.reciprocal(rz[:], zp[:, 0:1])
                rp = ps_o.tile([P, H], F32, tag="o", name=f"o{br}b{b}")
                nc.tensor.matmul(rp[:], e_t[:, b, :], nat_other[:, b * H:(b + 1) * H],
                                 start=True, stop=True)
                ot = work.tile([P, H], F32, tag="ot", bufs=2)
                nc.vector.scalar_tensor_tensor(
                    ot[:], rp[:], rz[:, 0:1], nat_self[:, b * H:(b + 1) * H].bitcast(F32),
                    MULT, ADD)
                nc.sync.dma_start(out=out_d.ap()[b * P:(b + 1) * P, :],
                                  in_=ot[:])

            e_t0 = work.tile([P, BL, S], F32R, tag="e0", bufs=1)
            e_t1 = work.tile([P, BL, S], F32R, tag="e1", bufs=1)

            # ================= branch 0 (aspect) ========================
            for k in stage1(0):
                if k == 5:
                    stage2_ck(0, xp_lhsT, 0)
                    sca0 = sca_all(vbc_a)
            load_g_head(g_pa_d)     # prefetch; WAR-gated on br0-s1 reads
            nc.sync.dma_start(out=xa_nat[:], in_=xa_nat_d.ap()[:])
            for i in range(6):
                load_g_k(g_pa_d, i)
            stage2_ck(0, xp_lhsT, 1)
            # batched second half + exp (runs under br1-s1)
            scb0 = work.tile([P, BL, S], F32, tag="scb", bufs=1)
            nc.vector.tensor_scalar_mul(scb0[:], th_all[:, KC, :, :],
                                        vbc_a[:, KC:KC + 1])
            for j in range(1, KC):
                nc.vector.scalar_tensor_tensor(
                    scb0[:], th_all[:, KC + j, :, :],
                    vbc_a[:, KC + j:KC + j + 1], scb0[:], MULT, ADD)
            sc0 = work.tile([P, BL, S], F32, tag="sc", bufs=1)
            nc.vector.tensor_tensor(sc0[:], sca0[:], scb0[:], ADD)
            nc.scalar.activation(e_t0[:], sc0[:], Exp)

            # ================= branch 1 (polarity) ======================
            # br1 stage 1 with br1-s2-ck0 spliced at k=3 and br0's z/out
            # matmuls interleaved so the PE never waits on softmax chains.
            zo = 0
            for k in stage1(1):
                if k == 5:
                    stage2_ck(1, xa_lhsT, 0)
                    sca1 = sca_all(vbc_p)
                elif k in (2, 3, 4, 6):
                    zout(0, zo, e_t0, xp_nat, xa_nat, out_a_d)
                    zo += 1
            stage2_ck(1, xa_lhsT, 1)

            # Tail: pair-batched second-half chains on vector; the
            # scale+residual combine rides scalar (act-Copy-scale) + gpsimd
            # (tensor add) so no engine saturates after the last tanh.
            Copy = mybir.ActivationFunctionType.Copy
            for pr in range(2):
                bs2 = slice(2 * pr, 2 * pr + 2)
                scb = work.tile([P, 2, S], F32, tag=f"scb1_{pr}", bufs=1)
                nc.vector.tensor_scalar_mul(scb[:], th_all[:, KC, bs2, :],
                                            vbc_p[:, KC:KC + 1])
                for j in range(1, KC):
                    nc.vector.scalar_tensor_tensor(
                        scb[:], th_all[:, KC + j, bs2, :],
                        vbc_p[:, KC + j:KC + j + 1], scb[:], MULT, ADD)
                sc = work.tile([P, 2, S], F32, tag=f"sc1_{pr}", bufs=1)
                nc.vector.tensor_tensor(sc[:], sca1[:, bs2, :], scb[:], ADD)
                nc.scalar.activation(e_t1[:, bs2, :], sc[:], Exp)
                for b in (2 * pr, 2 * pr + 1):
                    zp = ps_z.tile([P, 2], F32, tag="z", name=f"z1b{b}")
                    nc.tensor.matmul(zp[:], e_t1[:, b, :], ones_t[:],
                                     start=True, stop=True)
                    rz = work.tile([P, 1], F32, tag="rz", bufs=4)
                    nc.vector.reciprocal(rz[:], zp[:, 0:1])
                    rp = ps_o.tile([P, H], F32, tag="o", name=f"o1b{b}")
                    nc.tensor.matmul(rp[:], e_t1[:, b, :], xa_nat[:, b * H:(b + 1) * H],
                                     start=True, stop=True)
                    if pr == 0:
                        # early pair: combine on scalar+gpsimd, leaving
                        # vector free for the late pair's chain
                        ots = work.tile([P, H], F32, tag="ots", bufs=2)
                        nc.scalar.activation(ots[:], rp[:], Copy,
                                             scale=rz[:, 0:1])
                        otf = work.tile([P, H], F32, tag="otf", bufs=2)
                        nc.gpsimd.tensor_tensor(
                            otf[:], ots[:],
                            xp_nat[:, b * H:(b + 1) * H].bitcast(F32), ADD)
                    else:
                        # late pair: vector is idle by now — single stt
                        otf = work.tile([P, H], F32, tag="otf", bufs=2)
                        nc.vector.scalar_tensor_tensor(
                            otf[:], rp[:], rz[:, 0:1],
                            xp_nat[:, b * H:(b + 1) * H].bitcast(F32),
                            MULT, ADD)
                    nc.sync.dma_start(
                        out=out_p_d.ap()[b * P:(b + 1) * P, :], in_=otf[:])

    nc.compile()
    return nc


def _get_nc():
    if "nc" not in _cache:
        _cache["nc"] = _build()
    return _cache["nc"]


def _prep_in_maps(aspect_hidden, polarity_hidden, G_aspect_polarity,
                  G_polarity_aspect, G_vector_aspect, G_vector_polarity):
    f = np.float32

    def shuffle_g(g):
        # host-side image of the SBUF G piece tiles, concatenated in
        # consumption order: k0 per-h, k1, then per-k
        gr = np.asarray(g, dtype=f).reshape(HT, P, K, H)
        pieces = [gr[h, :, 0, :] for h in range(HT)]
        pieces.append(gr[:, :, 1, :].transpose(1, 0, 2).reshape(P, HT * H))
        for k in range(2, K):
            pieces.append(
                gr[:, :, k, :].transpose(1, 0, 2).reshape(P, HT * H))
        return np.ascontiguousarray(np.concatenate(pieces, axis=1))

    def shuffle_t(x_loc):
        # [BS,H] -> transposed partition-major [P, (ht, bs)]
        return np.ascontiguousarray(
            x_loc.T.reshape(HT, P, BS).transpose(1, 0, 2))

    def shuffle_nat(x_loc):
        # [BS,H] -> partition-major [P, (b, h)]
        return np.ascontiguousarray(
            x_loc.reshape(BL, P, H).transpose(1, 0, 2).reshape(P, BL * H))

    a = np.ascontiguousarray(aspect_hidden, dtype=f)
    p = np.ascontiguousarray(polarity_hidden, dtype=f)
    g_ap = shuffle_g(G_aspect_polarity)
    g_pa = shuffle_g(G_polarity_aspect)
    v_ap = np.ascontiguousarray(G_vector_aspect, dtype=f)
    v_pa = np.ascontiguousarray(G_vector_polarity, dtype=f)

    in_maps = []
    for c in range(NCORES):
        a_loc = a[c * BL:(c + 1) * BL].reshape(BS, H)
        p_loc = p[c * BL:(c + 1) * BL].reshape(BS, H)
        m = {
            "xa_t": np.ascontiguousarray(shuffle_t(a_loc).reshape(P, HT * BS)),
            "xp_t": np.ascontiguousarray(shuffle_t(p_loc).reshape(P, HT * BS)),
            "xa_nat": shuffle_nat(a_loc),
            "xp_nat": shuffle_nat(p_loc),
            "g_ap": g_ap,
            "g_pa": g_pa,
            "v_ap": v_ap,
            "v_pa": v_pa,
        }
        in_maps.append(m)
    return in_maps


def kernel(aspect_hidden, polarity_hidden, G_aspect_polarity,
           G_polarity_aspect, G_vector_aspect, G_vector_polarity):
    nc = _get_nc()
    in_maps = _prep_in_maps(aspect_hidden, polarity_hidden, G_aspect_polarity,
                            G_polarity_aspect, G_vector_aspect,
                            G_vector_polarity)
    res = run_bass_kernel_spmd(
        nc, in_maps, core_ids=list(range(NCORES)),
        trace=bool(os.environ.get("KERNEL_TRACE")))
    _cache["last_results"] = res

    out_a = np.empty((B, S, H), np.float32)
    out_p = np.empty((B, S, H), np.float32)
    for c in range(NCORES):
        out_a[c * BL:(c + 1) * BL] = res.results[c]["out_a"].reshape(BL, S, H)
        out_p[c * BL:(c + 1) * BL] = res.results[c]["out_p"].reshape(BL, S, H)
    return (out_a, out_p)


# revision 20
# speedup vs baseline: 1.0184x; 1.0184x over previous
"""CrossSharedUnit Trainium2 kernel — 8-core data-parallel over batch.

Reference computation (per batch b, S=128 tokens, H=512 hidden, K=8):
  proj[b,s,k,g] = sum_h left[b,s,h] * G[h,k,g]
  raw[b,s,t,k]  = tanh(sum_g proj[b,s,k,g] * right[b,t,g])
  score[b,s,t]  = sum_k raw[b,s,t,k] * v[k]
  attn          = softmax(score, axis=t)
  out           = self + attn @ other_hidden
for two branches (aspect: left=aspect, right=polarity; polarity: left=aspect,
right=aspect — faithful to the source which uses aspect on BOTH sides).

Sharding: batch B=32 split 4-per-core across 8 cores; G tensors replicated.
No collectives.

Schedule: the PE (tensor engine) is the bottleneck (~78us of fp32r matmul),
so the program is one continuous PE stream:
  warmup | br0-s1 (+ br0-s2-ck0 spliced at k=3) | br0-s2-ck1
         | br1-s1 (+ br1-s2-ck0 splice + br0 z/out mms interleaved)
         | br1-s2-ck1 | br1 z/out
with softmax chains on vector/gpsimd/scalar underneath the next phase's
matmuls. All input DMA issues live on the sync queue in exact consumption
order (plus two tiny v-loads on gpsimd), so a blocked G prefetch (bufs=1
buffer recycling between branches) can never head-of-line-block a PSUM
evacuation; output stores ride the sync queue after the loads are done.

All matmuls are float32r (TF32-like, 1 row/cycle at free>=256). Softmax
needs no max-subtraction: |score| <= sum|v_k| so exp() cannot overflow in
fp32. The softmax division is deferred through the attention matmul:
out = self + (E @ other) / Z with Z from a ones-matmul.
"""

import os
import sys

sys.path.insert(0, "/opt/trn_rl_repo")

import numpy as np

from concourse import bacc, mybir, tile
from concourse.bass_utils import run_bass_kernel_spmd

B, S, H, K = 32, 128, 512, 8
NCORES = 8
BL = B // NCORES          # batches per core
BS = BL * S               # rows per core (512)
P = 128                   # partitions
HT = H // P               # h partition-tiles (4)
KG = K * H                # flattened (k,g) axis (4096)
KC = K // 2               # k's per stage-2 chunk (4)
F32 = mybir.dt.float32
F32R = mybir.dt.float32r

_cache = {}


def _build():
    """Build + compile the per-core Bass program (same program on all cores)."""
    nc = bacc.Bacc("TRN2", target_bir_lowering=False, debug=False,
                   num_devices=NCORES)

    xa_t_d = nc.dram_tensor("xa_t", [P, HT * BS], F32R, kind="ExternalInput")
    xp_t_d = nc.dram_tensor("xp_t", [P, HT * BS], F32R, kind="ExternalInput")
    xa_nat_d = nc.dram_tensor("xa_nat", [P, BL * H], F32R, kind="ExternalInput")
    xp_nat_d = nc.dram_tensor("xp_nat", [P, BL * H], F32R, kind="ExternalInput")
    # G: one pre-shuffled tensor per branch; pieces are column slices in
    # consumption order: 4x k0 per-h, 2x k1 halves, 6x per-k
    g_ap_d = nc.dram_tensor("g_ap", [P, HT * KG], F32R, kind="ExternalInput")
    g_pa_d = nc.dram_tensor("g_pa", [P, HT * KG], F32R, kind="ExternalInput")
    v_ap_d = nc.dram_tensor("v_ap", [K, 1], F32, kind="ExternalInput")
    v_pa_d = nc.dram_tensor("v_pa", [K, 1], F32, kind="ExternalInput")
    out_a_d = nc.dram_tensor("out_a", [BS, H], F32, kind="ExternalOutput")
    out_p_d = nc.dram_tensor("out_p", [BS, H], F32, kind="ExternalOutput")

    Tanh = mybir.ActivationFunctionType.Tanh
    Exp = mybir.ActivationFunctionType.Exp
    MULT = mybir.AluOpType.mult
    ADD = mybir.AluOpType.add

    with tile.TileContext(nc) as tc:
        with (
            tc.tile_pool(name="const", bufs=1) as cpool,
            tc.tile_pool(name="g", bufs=1) as gpool,
            tc.tile_pool(name="proj", bufs=1) as projpool,
            tc.tile_pool(name="work", bufs=2) as work,
            tc.tile_pool(name="ps_acc", bufs=4, space="PSUM") as ps_acc,
            tc.tile_pool(name="ps_o", bufs=2, space="PSUM") as ps_o,
            tc.tile_pool(name="ps_z", bufs=2, space="PSUM") as ps_z,
        ):
            # ---- constants + warmup weights (vector queue) --------------
            wm = cpool.tile([P, BS], F32R, tag="wm")
            nc.vector.memset(wm[:].bitcast(F32), 0.0)
            ones_t = cpool.tile([P, 2], F32R, tag="ones_t")
            nc.vector.memset(ones_t[:].bitcast(F32), 1.0)

            # ---- persistent activations --------------------------------
            xa_t0 = cpool.tile([P, BS], F32R, tag="xa_t0")
            xa_tb = cpool.tile([P, (HT - 1) * BS], F32R, tag="xa_tb")
            xp_t = cpool.tile([P, HT * BS], F32R, tag="xp_t")
            xa_nat = cpool.tile([P, BL * H], F32R, tag="xa_nat")
            xp_nat = cpool.tile([P, BL * H], F32R, tag="xp_nat")

            vrow_a = cpool.tile([1, K], F32, tag="vrow_a")
            vrow_p = cpool.tile([1, K], F32, tag="vrow_p")
            nc.gpsimd.dma_start(out=vrow_a[:], in_=v_ap_d.ap().rearrange("k o -> o k"))
            nc.gpsimd.dma_start(out=vrow_p[:], in_=v_pa_d.ap().rearrange("k o -> o k"))
            vbc_a = cpool.tile([P, K], F32, tag="vbc_a")
            vbc_p = cpool.tile([P, K], F32, tag="vbc_p")
            nc.gpsimd.partition_broadcast(vbc_a[:], vrow_a[:])
            nc.gpsimd.partition_broadcast(vbc_p[:], vrow_p[:])

            # ---- G piece tiles (shared bufs=1 between branches) ---------
            g_k0 = [gpool.tile([P, H], F32R, tag=f"g_k0_{h}",
                                name=f"g_k0_{h}") for h in range(HT)]
            g_k1a = gpool.tile([P, 2 * H], F32R, tag="g_k1a")
            g_k1b = gpool.tile([P, 2 * H], F32R, tag="g_k1b")
            g_kk = [gpool.tile([P, HT * H], F32R, tag=f"g_kk{i}",
                               name=f"g_kk{i}")
                     for i in range(6)]

            # ---- the loader. Host arrays are pre-shuffled partition-major
            # so every DMA is 128 descriptors of one big contiguous chunk.
            # The startup-critical pieces (k0 G + aspect) are 256KB each and
            # spread over three queues for DMA-engine concurrency.
            def load_g_head(g_d):
                for h in range(HT):
                    nc.sync.dma_start(out=g_k0[h][:],
                                      in_=g_d.ap()[:, h * H:(h + 1) * H])
                nc.sync.dma_start(out=g_k1a[:],
                                  in_=g_d.ap()[:, HT * H:HT * H + 2 * H])
                nc.sync.dma_start(out=g_k1b[:],
                                  in_=g_d.ap()[:, HT * H + 2 * H:2 * HT * H])

            def load_g_k(g_d, i):
                o = (2 + i) * HT * H
                nc.sync.dma_start(out=g_kk[i][:], in_=g_d.ap()[:, o:o + HT * H])

            # interleave xa-h / G-k0-h so the PE can start after 512KB and
            # gets one h-step per ~0.8us thereafter
            nc.sync.dma_start(out=xa_t0[:], in_=xa_t_d.ap()[:, 0:BS])
            nc.sync.dma_start(out=g_k0[0][:], in_=g_ap_d.ap()[:, 0:H])
            for h in range(1, HT):
                nc.sync.dma_start(out=xa_tb[:, (h - 1) * BS:h * BS],
                                  in_=xa_t_d.ap()[:, h * BS:(h + 1) * BS])
                nc.sync.dma_start(out=g_k0[h][:],
                                  in_=g_ap_d.ap()[:, h * H:(h + 1) * H])
            nc.sync.dma_start(out=g_k1a[:],
                              in_=g_ap_d.ap()[:, HT * H:HT * H + 2 * H])
            nc.sync.dma_start(out=g_k1b[:],
                              in_=g_ap_d.ap()[:, HT * H + 2 * H:2 * HT * H])
            for i in range(4):
                load_g_k(g_ap_d, i)
            nc.sync.dma_start(out=xp_t[:], in_=xp_t_d.ap()[:])
            load_g_k(g_ap_d, 4)
            load_g_k(g_ap_d, 5)
            nc.sync.dma_start(out=xp_nat[:], in_=xp_nat_d.ap()[:])
            # (g_pa loads are emitted after br0-s1 so the WAR deps pick up
            #  br0's reads; xa_nat after those.)

            def g_lhsT(k, h, gt):
                if k == 0:
                    return g_k0[h][:, gt * P:(gt + 1) * P]
                if k == 1:
                    piece = g_k1a if h < 2 else g_k1b
                    o = (h % 2) * H + gt * P
                    return piece[:, o:o + P]
                piece = g_kk[k - 2]
                o = h * H + gt * P
                return piece[:, o:o + P]

            def xa_rhs(h):
                if h == 0:
                    return xa_t0[:]
                return xa_tb[:, (h - 1) * BS:h * BS]

            def xa_lhsT(gi, b):
                if gi == 0:
                    return xa_t0[:, b * S:(b + 1) * S]
                o = (gi - 1) * BS + b * S
                return xa_tb[:, o:o + S]

            def xp_lhsT(gi, b):
                o = gi * BS + b * S
                return xp_t[:, o:o + S]

            # projT2[gt][g_part, k, b, s] — stage-1 output, stage-2 rhs.
            projT2 = [projpool.tile([P, K, BL, S], F32R, tag=f"projT2_{gt}",
                                    name=f"projT2_{gt}")
                      for gt in range(HT)]

            evac_state = [0]

            def evac(dst, src):
                # ping-pong PSUM evacuations between vector and scalar
                if evac_state[0] % 2 == 0:
                    nc.vector.tensor_copy(dst, src)
                else:
                    nc.scalar.copy(dst, src)
                evac_state[0] += 1

            # ---- PE warmup: get the p-state ramp going during DMA lead-in
            for w in range(6):
                acc = ps_acc.tile([P, BL, S], F32, tag="acc", name=f"warm{w}")
                nc.tensor.matmul(acc[:], wm[:, 0:P], wm[:],
                                 start=True, stop=True)

            def stage1(br):
                # k0 h-outer with 4 open accumulators: first matmuls need
                # only g_k0h0 + xa_t0 (512KB total).
                accs = [ps_acc.tile([P, BL, S], F32, tag="acc",
                                    name=f"s1a{br}k0g{gt}")
                        for gt in range(HT)]
                for h in range(HT):
                    for gt in range(HT):
                        nc.tensor.matmul(
                            accs[gt][:], g_lhsT(0, h, gt), xa_rhs(h),
                            start=(h == 0), stop=(h == HT - 1),
                            skip_group_check=True)
                for gt in range(HT):
                    evac(projT2[gt][:, 0, :, :], accs[gt][:])
                for k in range(1, K):
                    for gt in range(HT):
                        acc = ps_acc.tile([P, BL, S], F32, tag="acc",
                                          name=f"s1a{br}k{k}g{gt}")
                        for h in range(HT):
                            nc.tensor.matmul(
                                acc[:], g_lhsT(k, h, gt), xa_rhs(h),
                                start=(h == 0), stop=(h == HT - 1))
                        evac(projT2[gt][:, k, :, :], acc[:])
                    yield k

            # th_all[t_part, k, b, s]: tanh(stage-2) output, both branches
            # (WAR-recycled). Score ops slice [:, j, :, :] batched over b.
            th_all = work.tile([P, K, BL, S], F32, tag="th", bufs=1)

            def stage2_ck(br, lhsT_of, ck):
                # raw[t, k, s] = tanh(sum_g right[t,g] proj[g,k,s]) per batch
                for b in range(BL):
                    acc2 = ps_acc.tile([P, KC, S], F32, tag="acc",
                                       name=f"s2a{br}b{b}c{ck}")
                    for gi in range(HT):
                        nc.tensor.matmul(
                            acc2[:],
                            lhsT_of(gi, b),
                            projT2[gi][:, ck * KC:(ck + 1) * KC, b, :],
                            start=(gi == 0), stop=(gi == HT - 1))
                    nc.scalar.activation(
                        th_all[:, ck * KC:(ck + 1) * KC, b, :], acc2[:], Tanh)

            def sca_all(vbc):
                # first-half score partial, batched over all 4 batches
                sca = work.tile([P, BL, S], F32, tag="sca", bufs=1)
                nc.vector.tensor_scalar_mul(sca[:], th_all[:, 0, :, :],
                                            vbc[:, 0:1])
                for j in range(1, KC):
                    nc.vector.scalar_tensor_tensor(
                        sca[:], th_all[:, j, :, :], vbc[:, j:j + 1], sca[:],
                        MULT, ADD)
                return sca

            def zout(br, b, e_t, nat_other, nat_self, out_d):
                # out = self + (E_T.T @ other) / Z, Z via ones-matmul.
                zp = ps_z.tile([P, 2], F32, tag="z", name=f"z{br}b{b}")
                nc.tensor.matmul(zp[:], e_t[:, b, :], ones_t[:],
                                 start=True, stop=True)
                rz = work.tile([P, 1], F32, tag="rz", bufs=4)
                nc.vector.reciprocal(rz[:], zp[:, 0:1])
                rp = ps_o.tile([P, H], F32, tag="o", name=f"o{br}b{b}")
                nc.tensor.matmul(rp[:], e_t[:, b, :], nat_other[:, b * H:(b + 1) * H],
                                 start=True, stop=True)
                ot = work.tile([P, H], F32, tag="ot", bufs=2)
                nc.vector.scalar_tensor_tensor(
                    ot[:], rp[:], rz[:, 0:1], nat_self[:, b * H:(b + 1) * H].bitcast(F32),
                    MULT, ADD)
                nc.sync.dma_start(out=out_d.ap()[b * P:(b + 1) * P, :],
                                  in_=ot[:])

            e_t0 = work.tile([P, BL, S], F32R, tag="e0", bufs=1)
            e_t1 = work.tile([P, BL, S], F32R, tag="e1", bufs=1)

            # ================= branch 0 (aspect) ========================
            for k in stage1(0):
                if k == 5:
                    stage2_ck(0, xp_lhsT, 0)
                    sca0 = sca_all(vbc_a)
            load_g_head(g_pa_d)     # prefetch; WAR-gated on br0-s1 reads
            nc.sync.dma_start(out=xa_nat[:], in_=xa_nat_d.ap()[:])
            for i in range(6):
                load_g_k(g_pa_d, i)
            stage2_ck(0, xp_lhsT, 1)
            # batched second half + exp (runs under br1-s1)
            scb0 = work.tile([P, BL, S], F32, tag="scb", bufs=1)
            nc.vector.tensor_scalar_mul(scb0[:], th_all[:, KC, :, :],
                                        vbc_a[:, KC:KC + 1])
            for j in range(1, KC):
                nc.vector.scalar_tensor_tensor(
                    scb0[:], th_all[:, KC + j, :, :],
                    vbc_a[:, KC + j:KC + j + 1], scb0[:], MULT, ADD)
            nc.vector.tensor_tensor(scb0[:], sca0[:], scb0[:], ADD)
            nc.scalar.activation(e_t0[:], scb0[:], Exp)

            # ================= branch 1 (polarity) ======================
            # br1 stage 1 with br1-s2-ck0 spliced at k=3 and br0's z/out
            # matmuls interleaved so the PE never waits on softmax chains.
            zo = 0
            for k in stage1(1):
                if k == 5:
                    stage2_ck(1, xa_lhsT, 0)
                    sca1 = sca_all(vbc_p)
                elif k in (2, 3, 4, 6):
                    zout(0, zo, e_t0, xp_nat, xa_nat, out_a_d)
                    zo += 1
            stage2_ck(1, xa_lhsT, 1)

            # Tail: pair-batched second-half chains on vector; the
            # scale+residual combine rides scalar (act-Copy-scale) + gpsimd
            # (tensor add) so no engine saturates after the last tanh.
            Copy = mybir.ActivationFunctionType.Copy
            for pr in range(2):
                bs2 = slice(2 * pr, 2 * pr + 2)
                scb = work.tile([P, 2, S], F32, tag=f"scb1_{pr}", bufs=1)
                nc.vector.tensor_scalar_mul(scb[:], th_all[:, KC, bs2, :],
                                            vbc_p[:, KC:KC + 1])
                for j in range(1, KC):
                    nc.vector.scalar_tensor_tensor(
                        scb[:], th_all[:, KC + j, bs2, :],
                        vbc_p[:, KC + j:KC + j + 1], scb[:], MULT, ADD)
                nc.vector.tensor_tensor(scb[:], sca1[:, bs2, :], scb[:], ADD)
                nc.scalar.activation(e_t1[:, bs2, :], scb[:], Exp)
                for b in (2 * pr, 2 * pr + 1):
                    zp = ps_z.tile([P, 2], F32, tag="z", name=f"z1b{b}")
                    nc.tensor.matmul(zp[:], e_t1[:, b, :], ones_t[:],
                                     start=True, stop=True)
                    rz = work.tile([P, 1], F32, tag="rz", bufs=4)
                    nc.vector.reciprocal(rz[:], zp[:, 0:1])
                    rp = ps_o.tile([P, H], F32, tag="o", name=f"o1b{b}")
                    nc.tensor.matmul(rp[:], e_t1[:, b, :], xa_nat[:, b * H:(b + 1) * H],
                                     start=True, stop=True)
                    if pr == 0:
                        # early pair: combine on scalar+gpsimd, leaving
                        # vector free for the late pair's chain
                        ots = work.tile([P, H], F32, tag="ots", bufs=2)
                        nc.scalar.activation(ots[:], rp[:], Copy,
                                             scale=rz[:, 0:1])
                        otf = work.tile([P, H], F32, tag="otf", bufs=2)
                        nc.gpsimd.tensor_tensor(
                            otf[:], ots[:],
                            xp_nat[:, b * H:(b + 1) * H].bitcast(F32), ADD)
                    else:
                        # late pair: vector is idle by now — single stt
                        otf = work.tile([P, H], F32, tag="otf_v", bufs=2)
                        nc.vector.scalar_tensor_tensor(
                            otf[:], rp[:], rz[:, 0:1],
                            xp_nat[:, b * H:(b + 1) * H].bitcast(F32),
                            MULT, ADD)
                    nc.sync.dma_start(
                        out=out_p_d.ap()[b * P:(b + 1) * P, :], in_=otf[:])

    nc.compile()
    return nc


def _get_nc():
    if "nc" not in _cache:
        _cache["nc"] = _build()
    return _cache["nc"]


def _prep_in_maps(aspect_hidden, polarity_hidden, G_aspect_polarity,
                  G_polarity_aspect, G_vector_aspect, G_vector_polarity):
    f = np.float32

    def shuffle_g(g):
        # host-side image of the SBUF G piece tiles, concatenated in
        # consumption order: k0 per-h, k1, then per-k
        gr = np.asarray(g, dtype=f).reshape(HT, P, K, H)
        pieces = [gr[h, :, 0, :] for h in range(HT)]
        pieces.append(gr[:, :, 1, :].transpose(1, 0, 2).reshape(P, HT * H))
        for k in range(2, K):
            pieces.append(
                gr[:, :, k, :].transpose(1, 0, 2).reshape(P, HT * H))
        return np.ascontiguousarray(np.concatenate(pieces, axis=1))

    def shuffle_t(x_loc):
        # [BS,H] -> transposed partition-major [P, (ht, bs)]
        return np.ascontiguousarray(
            x_loc.T.reshape(HT, P, BS).transpose(1, 0, 2))

    def shuffle_nat(x_loc):
        # [BS,H] -> partition-major [P, (b, h)]
        return np.ascontiguousarray(
            x_loc.reshape(BL, P, H).transpose(1, 0, 2).reshape(P, BL * H))

    a = np.ascontiguousarray(aspect_hidden, dtype=f)
    p = np.ascontiguousarray(polarity_hidden, dtype=f)
    g_ap = shuffle_g(G_aspect_polarity)
    g_pa = shuffle_g(G_polarity_aspect)
    v_ap = np.ascontiguousarray(G_vector_aspect, dtype=f)
    v_pa = np.ascontiguousarray(G_vector_polarity, dtype=f)

    in_maps = []
    for c in range(NCORES):
        a_loc = a[c * BL:(c + 1) * BL].reshape(BS, H)
        p_loc = p[c * BL:(c + 1) * BL].reshape(BS, H)
        m = {
            "xa_t": np.ascontiguousarray(shuffle_t(a_loc).reshape(P, HT * BS)),
            "xp_t": np.ascontiguousarray(shuffle_t(p_loc).reshape(P, HT * BS)),
            "xa_nat": shuffle_nat(a_loc),
            "xp_nat": shuffle_nat(p_loc),
            "g_ap": g_ap,
            "g_pa": g_pa,
            "v_ap": v_ap,
            "v_pa": v_pa,
        }
        in_maps.append(m)
    return in_maps


def kernel(aspect_hidden, polarity_hidden, G_aspect_polarity,
           G_polarity_aspect, G_vector_aspect, G_vector_polarity):
    nc = _get_nc()
    in_maps = _prep_in_maps(aspect_hidden, polarity_hidden, G_aspect_polarity,
                            G_polarity_aspect, G_vector_aspect,
                            G_vector_polarity)
    res = run_bass_kernel_spmd(
        nc, in_maps, core_ids=list(range(NCORES)),
        trace=bool(os.environ.get("KERNEL_TRACE")))
    _cache["last_results"] = res

    out_a = np.empty((B, S, H), np.float32)
    out_p = np.empty((B, S, H), np.float32)
    for c in range(NCORES):
        out_a[c * BL:(c + 1) * BL] = res.results[c]["out_a"].reshape(BL, S, H)
        out_p[c * BL:(c + 1) * BL] = res.results[c]["out_p"].reshape(BL, S, H)
    return (out_a, out_p)
